# revision 11
# baseline (speedup 1.0000x reference)
"""Trainium2 Bass kernel for the seasonal-decomposition block (fp8 DoubleRow).

Math: for each season s, circ_s = real(F_s^H diag(d_s) F_s) is a symmetric
Toeplitz matrix whose first column c_s(t) is computed on host with one
length-N FFT. Every 128x128 block of circ_s is a contiguous 128-column
slice of the skewed buffer E2F_s[p, m] = c_s(|1920 + p - m|) ([128, 3968]),
so the LxL matrix is never materialized.

The recurrence  x_rem <- x_rem - tanh(x_rem @ circ_s)  runs in transposed
layout (positions on partitions, rows on the free axis). Matmuls use fp8
(e4m3) operands with MatmulPerfMode.DoubleRow: each instruction contracts
two adjacent 128-blocks at double rate. Adjacent position-chunks are
pair-swapped in the x layout (position j holds logical chunk j^1) so a
single [128, 2, RPC] access pattern pairs with two adjacent weight blocks
of the skew buffer, which are contiguous by construction.

State: x_rem is never materialized. Using x_rem_s = x - corr_s (with
corr_s = sum_{r<s} tanh_r, kept in bf16), the fp8 PE operand for the next
season is one fused DVE op  xr8 = fp8(x_bf - corr)  per chunk-pair, and
the trailing avg-pool trend is  T.T @ x_bf - T.T @ corr  accumulated in
one PSUM group via positive and negated bf16 band matrices (fp8 bands
would corrupt the replicate-pad coefficients). Weights are scaled by 256
on host before fp8 quantization (entries ~3e-3 would be subnormal) and
descaled inside the tanh activation (scale=1/256). Emulated end-to-end
rel_err vs the fp64 reference: 1.20e-2 (gate: 2e-2).

Sharding: pure data-parallel over the B*C = 2048 rows, 256 rows per core,
8 cores, no collectives.
"""

import sys

sys.path.insert(0, "/opt/trn_rl_repo")

import numpy as np
import ml_dtypes

import concourse.mybir as mybir
import concourse.tile as tile
from concourse import bacc
from concourse.bass_utils import run_bass_kernel_spmd

L = 2048
S = 4
NFULL = L * S
KER = 25
B, C = 64, 32
NCORES = 8
ROWS = B * C          # 2048
RPC = ROWS // NCORES  # 256 rows per core
NCHUNK = L // 128     # 16
WSCALE = 256.0

_f32 = mybir.dt.float32
_f8 = mybir.dt.float8e4
_bf16 = mybir.dt.bfloat16
_np_f8 = ml_dtypes.float8_e4m3
_np_bf16 = ml_dtypes.bfloat16
_DR = mybir.MatmulPerfMode.DoubleRow


def _build_tband():
    """Three [128,128] band blocks of the avg-pool matrix T (trend = T.T @ x)."""
    u = np.arange(128)[:, None]
    t = np.arange(128)[None, :]
    diag = ((t - u >= 0) & (t - u <= KER - 1)).astype(np.float32) / KER
    sub = ((u - t) >= 128 - (KER - 1)).astype(np.float32) / KER
    t00 = diag.copy()
    t00[0, :] += np.maximum(0, (KER - 1) - np.arange(128)).astype(np.float32) / KER
    return np.ascontiguousarray(np.stack([t00, diag, sub], axis=1))  # [128, 3, 128]


_TBAND = _build_tband()
# skew index: E2F[p, m] = c(|1920 + p - m|), block(d) at cols [1920-128d, 2048-128d)
_E2F_IDX = np.abs(1920 + np.arange(128)[:, None] - np.arange(31 * 128)[None, :])
# pair-swap permutation: position j holds logical chunk j^1
_PERM = np.arange(NCHUNK) ^ 1


def _circ_cols(diagonals):
    """First columns c_s(t), t = 0..L-1, of each season's Toeplitz circ_s."""
    d = np.zeros((S, NFULL), dtype=np.float64)
    d[:, :L] = np.asarray(diagonals, dtype=np.float64)
    F = np.fft.fft(d, axis=1)
    t = np.arange(L)
    ph = np.exp((2j * np.pi / NFULL) * (np.arange(S)[:, None] * L * t[None, :]))
    return ((ph * np.conj(F[:, :L])).real / NFULL).astype(np.float32)  # [S, L]


def _emit_body(nc, pools, xr8_d, xbf_d, e2_d, tb_d, out_d):
    constp, xrp, workp, psum_a, psum_t = pools
    tanh_f = mybir.ActivationFunctionType.Tanh

    # ---- SBUF tiles + prologue DMA (ordered by first use) ----
    e2_sb = [constp.tile([128, 31, 128], _f8, tag=f"e2_{s}", name=f"e2_{s}") for s in range(S)]
    xr8_0 = constp.tile([128, NCHUNK, RPC], _f8, tag="xr8_0", name="xr8_0")
    xbf_0 = constp.tile([128, NCHUNK, RPC], _bf16, tag="xbf_0", name="xbf_0")
    tb_sb = constp.tile([128, 6, 128], _bf16, tag="tb")
    corr = constp.tile([128, NCHUNK, RPC], _bf16, tag="corr", name="corr")
    big_ob = constp.tile([128, NCHUNK, RPC], _f32, tag="bigob")

    # season-0 chain b=0 runs k ascending with rhs pair p=7 first: its first
    # matmul needs only x positions 14:16 + weights k=0,1. Fine-grained
    # pieces give early semaphores so the PE starts ~128 KB into the DMA.
    nc.sync.dma_start(xr8_0[:, 14:16, :], xr8_d[:, 14:16, :])
    nc.sync.dma_start(e2_sb[0][:, 0:2, :], e2_d[0][:, 0:2, :])
    nc.sync.dma_start(xr8_0[:, 8:14, :], xr8_d[:, 8:14, :])
    nc.sync.dma_start(e2_sb[0][:, 2:8, :], e2_d[0][:, 2:8, :])
    nc.sync.dma_start(xr8_0[:, 0:8, :], xr8_d[:, 0:8, :])
    nc.sync.dma_start(e2_sb[0][:, 8:16, :], e2_d[0][:, 8:16, :])
    nc.sync.dma_start(e2_sb[0][:, 16:31, :], e2_d[0][:, 16:31, :])
    nc.sync.dma_start(xbf_0[:], xbf_d[:])
    for s in range(1, S):
        nc.sync.dma_start(e2_sb[s][:], e2_d[s])
    nc.sync.dma_start(tb_sb[:], tb_d[:])

    xr8_cur = xr8_0

    for s in range(S):
        last = s == S - 1
        xr8_nxt = None if last else xrp.tile(
            [128, NCHUNK, RPC], _f8, tag="xr8n", name=f"xr8n{s}")
        for b in range(NCHUNK):
            q = b // 2
            if b % 2 == 0:
                acc = psum_a.tile([128, 2, RPC], _f32, tag="acc")
            # season 0: ascending k (weight DMA arrival order);
            # later seasons: ascending pair (x written pair-by-pair)
            p_order = range(7, -1, -1) if s == 0 else range(8)
            for i, p in enumerate(p_order):
                k = b + 14 - 2 * p
                # chunk b lives at position b^1; within the pair that's 1-b%2
                nc.tensor.matmul(
                    acc[:, 1 - b % 2, :],
                    e2_sb[s][:, k : k + 2, :],
                    xr8_cur[:, 2 * p : 2 * p + 2, :],
                    start=(i == 0),
                    stop=(i == 7),
                    perf_mode=_DR,
                )
            if b % 2 == 1:
                sl = slice(2 * q, 2 * q + 2)
                # season 0: tanh lands straight in corr (corr = t); later
                # seasons: tanh to a scratch pair then corr += t. Then the
                # fused  xr8 = fp8(x - corr)  subcast. All on Act + DVE.
                if s == 0:
                    nc.scalar.activation(corr[:, sl, :], acc[:], tanh_f,
                                         scale=1.0 / WSCALE)
                else:
                    t_pair = workp.tile([128, 2, RPC], _bf16, tag="tp",
                                        name=f"tp{s}_{q}")
                    nc.scalar.activation(t_pair[:], acc[:], tanh_f,
                                         scale=1.0 / WSCALE)
                    nc.vector.tensor_add(out=corr[:, sl, :], in0=corr[:, sl, :],
                                         in1=t_pair[:])
                if not last:
                    nc.vector.tensor_sub(out=xr8_nxt[:, sl, :], in0=xbf_0[:, sl, :],
                                         in1=corr[:, sl, :])
        xr8_cur = xr8_nxt

    # ---- trend = T.T @ (x - corr) in one PSUM group; out = corr + trend ----
    # big_ob is kept in position order (chunk j^1 at index j); host unswaps.
    for j in range(NCHUNK):
        if j % 2 == 0:
            tps = psum_t.tile([128, 2, RPC], _f32, tag="tps", name=f"tps{j//2}")
        sl1 = 1 - j % 2  # position of logical chunk j within its pair
        if j == 0:
            nc.tensor.matmul(tps[:, sl1, :], tb_sb[:, 0, :], xbf_0[:, 1, :],
                             start=True, stop=False)
            nc.tensor.matmul(tps[:, sl1, :], tb_sb[:, 3, :], corr[:, 1, :],
                             start=False, stop=True)
        else:
            nc.tensor.matmul(tps[:, sl1, :], tb_sb[:, 2, :], xbf_0[:, (j - 1) ^ 1, :],
                             start=True, stop=False)
            nc.tensor.matmul(tps[:, sl1, :], tb_sb[:, 1, :], xbf_0[:, j ^ 1, :],
                             start=False, stop=False)
            nc.tensor.matmul(tps[:, sl1, :], tb_sb[:, 5, :], corr[:, (j - 1) ^ 1, :],
                             start=False, stop=False)
            nc.tensor.matmul(tps[:, sl1, :], tb_sb[:, 4, :], corr[:, j ^ 1, :],
                             start=False, stop=True)
        if j % 2 == 1:
            sl = slice(j - 1, j + 1)
            nc.vector.tensor_add(out=big_ob[:, sl, :], in0=corr[:, sl, :],
                                 in1=tps[:])
            nc.sync.dma_start(out_d[:, sl, :], big_ob[:, sl, :])


def build_nc(reps=1):
    nc = bacc.Bacc("TRN2", target_bir_lowering=False, debug=False)
    xr8_d = nc.dram_tensor("xr8", [128, NCHUNK, RPC], _f8, kind="ExternalInput")
    xbf_d = nc.dram_tensor("xbf", [128, NCHUNK, RPC], _bf16, kind="ExternalInput")
    e2_d = nc.dram_tensor("e2", [S, 128, 31, 128], _f8, kind="ExternalInput")
    tb_d = nc.dram_tensor("tb", [128, 6, 128], _bf16, kind="ExternalInput")
    out_d = nc.dram_tensor("out", [128, NCHUNK, RPC], _f32, kind="ExternalOutput")

    with tile.TileContext(nc) as tc:
        with (
            tc.tile_pool(name="const", bufs=1) as constp,
            tc.tile_pool(name="xrp", bufs=2) as xrp,
            tc.tile_pool(name="work", bufs=6) as workp,
            tc.tile_pool(name="psum_a", bufs=5, space="PSUM") as psum_a,
            tc.tile_pool(name="psum_t", bufs=2, space="PSUM") as psum_t,
        ):
            pools = (constp, xrp, workp, psum_a, psum_t)
            if reps == 1:
                _emit_body(nc, pools, xr8_d, xbf_d, e2_d, tb_d, out_d)
            else:
                with tc.For_i(0, reps, 1, staggered_reset=True,
                              hint_engines=(mybir.EngineType.PE,)):
                    _emit_body(nc, pools, xr8_d, xbf_d, e2_d, tb_d, out_d)
    nc.compile()
    return nc


_NC_CACHE = {}


def _get_nc(reps=1):
    if reps not in _NC_CACHE:
        _NC_CACHE[reps] = build_nc(reps)
    return _NC_CACHE[reps]


def make_in_maps(x, diagonals):
    c = _circ_cols(diagonals)
    e2 = (c * WSCALE)[:, _E2F_IDX].astype(_np_f8).reshape(S, 128, 31, 128)
    tb = np.concatenate([_TBAND, -_TBAND], axis=1).astype(_np_bf16)  # [128, 6, 128]
    xT = np.asarray(x, dtype=np.float32).reshape(ROWS, L).T  # [L, ROWS]
    in_maps = []
    for i in range(NCORES):
        xs = xT[:, i * RPC : (i + 1) * RPC].reshape(NCHUNK, 128, RPC)
        xs = np.ascontiguousarray(xs[_PERM].transpose(1, 0, 2))  # [128, 16, RPC]
        in_maps.append({
            "xr8": xs.astype(_np_f8),
            "xbf": xs.astype(_np_bf16),
            "e2": e2,
            "tb": tb,
        })
    return in_maps


def gather_out(results):
    parts = []
    for r in results:
        o = r["out"]  # [128, NCHUNK(position order), RPC]
        parts.append(np.ascontiguousarray(
            o.transpose(1, 0, 2)[_PERM]).reshape(L, RPC))
    outT = np.concatenate(parts, axis=1)  # [L, ROWS]
    return np.ascontiguousarray(outT.T).reshape(B, C, L).astype(np.float32)


def kernel(x, diagonals):
    x = np.asarray(x, dtype=np.float32)
    assert x.shape == (B, C, L) and np.asarray(diagonals).shape == (S, L)
    nc = _get_nc(1)
    in_maps = make_in_maps(x, diagonals)
    last_err = None
    for attempt in range(3):
        try:
            res = run_bass_kernel_spmd(nc, in_maps, core_ids=list(range(NCORES)))
            return gather_out(res.results)
        except Exception as ex:  # transient device errors
            last_err = ex
            import time as _time

            _time.sleep(2.0 * (attempt + 1))
    raise last_err


# revision 12
# speedup vs baseline: 1.0098x; 1.0098x over previous
"""Trainium2 Bass kernel for the seasonal-decomposition block (fp8 DoubleRow).

Math: for each season s, circ_s = real(F_s^H diag(d_s) F_s) is a symmetric
Toeplitz matrix whose first column c_s(t) is computed on host with one
length-N FFT. Every 128x128 block of circ_s is a contiguous 128-column
slice of the skewed buffer E2F_s[p, m] = c_s(|1920 + p - m|) ([128, 3968]),
so the LxL matrix is never materialized.

The recurrence  x_rem <- x_rem - tanh(x_rem @ circ_s)  runs in transposed
layout (positions on partitions, rows on the free axis). Matmuls use fp8
(e4m3) operands with MatmulPerfMode.DoubleRow: each instruction contracts
two adjacent 128-blocks at double rate. Adjacent position-chunks are
pair-swapped in the x layout (position j holds logical chunk j^1) so a
single [128, 2, RPC] access pattern pairs with two adjacent weight blocks
of the skew buffer, which are contiguous by construction.

State: x_rem is never materialized. Using x_rem_s = x - corr_s (with
corr_s = sum_{r<s} tanh_r, kept in bf16), the fp8 PE operand for the next
season is one fused DVE op  xr8 = fp8(x_bf - corr)  per chunk-pair, and
the trailing avg-pool trend is  T.T @ x_bf - T.T @ corr  accumulated in
one PSUM group via positive and negated bf16 band matrices (fp8 bands
would corrupt the replicate-pad coefficients). Weights are scaled by 256
on host before fp8 quantization (entries ~3e-3 would be subnormal) and
descaled inside the tanh activation (scale=1/256). Emulated end-to-end
rel_err vs the fp64 reference: 1.20e-2 (gate: 2e-2).

Sharding: pure data-parallel over the B*C = 2048 rows, 256 rows per core,
8 cores, no collectives.
"""

import sys

sys.path.insert(0, "/opt/trn_rl_repo")

import numpy as np
import ml_dtypes

import concourse.mybir as mybir
import concourse.tile as tile
from concourse import bacc
from concourse.bass_utils import run_bass_kernel_spmd

L = 2048
S = 4
NFULL = L * S
KER = 25
B, C = 64, 32
NCORES = 8
ROWS = B * C          # 2048
RPC = ROWS // NCORES  # 256 rows per core
NCHUNK = L // 128     # 16
WSCALE = 256.0

_f32 = mybir.dt.float32
_f8 = mybir.dt.float8e4
_bf16 = mybir.dt.bfloat16
_np_f8 = ml_dtypes.float8_e4m3
_np_bf16 = ml_dtypes.bfloat16
_DR = mybir.MatmulPerfMode.DoubleRow


def _build_tband():
    """Three [128,128] band blocks of the avg-pool matrix T (trend = T.T @ x)."""
    u = np.arange(128)[:, None]
    t = np.arange(128)[None, :]
    diag = ((t - u >= 0) & (t - u <= KER - 1)).astype(np.float32) / KER
    sub = ((u - t) >= 128 - (KER - 1)).astype(np.float32) / KER
    t00 = diag.copy()
    t00[0, :] += np.maximum(0, (KER - 1) - np.arange(128)).astype(np.float32) / KER
    return np.ascontiguousarray(np.stack([t00, diag, sub], axis=1))  # [128, 3, 128]


_TBAND = _build_tband()
# skew index: E2F[p, m] = c(|1920 + p - m|), block(d) at cols [1920-128d, 2048-128d)
_E2F_IDX = np.abs(1920 + np.arange(128)[:, None] - np.arange(31 * 128)[None, :])
# pair-swap permutation: position j holds logical chunk j^1
_PERM = np.arange(NCHUNK) ^ 1


def _circ_cols(diagonals):
    """First columns c_s(t), t = 0..L-1, of each season's Toeplitz circ_s."""
    d = np.zeros((S, NFULL), dtype=np.float64)
    d[:, :L] = np.asarray(diagonals, dtype=np.float64)
    F = np.fft.fft(d, axis=1)
    t = np.arange(L)
    ph = np.exp((2j * np.pi / NFULL) * (np.arange(S)[:, None] * L * t[None, :]))
    return ((ph * np.conj(F[:, :L])).real / NFULL).astype(np.float32)  # [S, L]


def _emit_body(nc, pools, xr8_d, xbf_d, e2_d, tb_d, out_d):
    constp, xrp, workp, psum_a, psum_t = pools
    tanh_f = mybir.ActivationFunctionType.Tanh

    # ---- SBUF tiles + prologue DMA (ordered by first use) ----
    e2_sb = [constp.tile([128, 31, 128], _f8, tag=f"e2_{s}", name=f"e2_{s}") for s in range(S)]
    xr8_0 = constp.tile([128, NCHUNK, RPC], _f8, tag="xr8_0", name="xr8_0")
    xbf_0 = constp.tile([128, NCHUNK, RPC], _bf16, tag="xbf_0", name="xbf_0")
    tb_sb = constp.tile([128, 6, 128], _bf16, tag="tb")
    corr = constp.tile([128, NCHUNK, RPC], _bf16, tag="corr", name="corr")
    big_ob = constp.tile([128, NCHUNK, RPC], _f32, tag="bigob")

    # season-0 chain b=0 runs k ascending with rhs pair p=7 first
    nc.sync.dma_start(xr8_0[:], xr8_d[:])
    nc.sync.dma_start(e2_sb[0][:, 0:16, :], e2_d[0][:, 0:16, :])
    nc.sync.dma_start(e2_sb[0][:, 16:31, :], e2_d[0][:, 16:31, :])
    nc.sync.dma_start(xbf_0[:], xbf_d[:])
    for s in range(1, S):
        nc.sync.dma_start(e2_sb[s][:], e2_d[s])
    nc.sync.dma_start(tb_sb[:], tb_d[:])

    xr8_cur = xr8_0

    for s in range(S):
        last = s == S - 1
        xr8_nxt = None if last else xrp.tile(
            [128, NCHUNK, RPC], _f8, tag="xr8n", name=f"xr8n{s}")
        for b in range(NCHUNK):
            q = b // 2
            if b % 2 == 0:
                acc = psum_a.tile([128, 2, RPC], _f32, tag="acc")
            # season 0: ascending k (weight DMA arrival order);
            # later seasons: ascending pair (x written pair-by-pair)
            p_order = range(7, -1, -1) if s == 0 else range(8)
            for i, p in enumerate(p_order):
                k = b + 14 - 2 * p
                # chunk b lives at position b^1; within the pair that's 1-b%2
                nc.tensor.matmul(
                    acc[:, 1 - b % 2, :],
                    e2_sb[s][:, k : k + 2, :],
                    xr8_cur[:, 2 * p : 2 * p + 2, :],
                    start=(i == 0),
                    stop=(i == 7),
                    perf_mode=_DR,
                )
            if b % 2 == 1:
                sl = slice(2 * q, 2 * q + 2)
                # season 0: tanh lands straight in corr (corr = t); later
                # seasons: tanh to a scratch pair then corr += t. Then the
                # fused  xr8 = fp8(x - corr)  subcast. All on Act + DVE.
                if s == 0:
                    nc.scalar.activation(corr[:, sl, :], acc[:], tanh_f,
                                         scale=1.0 / WSCALE)
                else:
                    t_pair = workp.tile([128, 2, RPC], _bf16, tag="tp",
                                        name=f"tp{s}_{q}")
                    nc.scalar.activation(t_pair[:], acc[:], tanh_f,
                                         scale=1.0 / WSCALE)
                    nc.vector.tensor_add(out=corr[:, sl, :], in0=corr[:, sl, :],
                                         in1=t_pair[:])
                if not last:
                    nc.vector.tensor_sub(out=xr8_nxt[:, sl, :], in0=xbf_0[:, sl, :],
                                         in1=corr[:, sl, :])
        xr8_cur = xr8_nxt

    # ---- trend = T.T @ (x - corr) in one PSUM group; out = corr + trend ----
    # big_ob is kept in position order (chunk j^1 at index j); host unswaps.
    for j in range(NCHUNK):
        if j % 2 == 0:
            tps = psum_t.tile([128, 2, RPC], _f32, tag="tps", name=f"tps{j//2}")
        sl1 = 1 - j % 2  # position of logical chunk j within its pair
        if j == 0:
            nc.tensor.matmul(tps[:, sl1, :], tb_sb[:, 0, :], xbf_0[:, 1, :],
                             start=True, stop=False)
            nc.tensor.matmul(tps[:, sl1, :], tb_sb[:, 3, :], corr[:, 1, :],
                             start=False, stop=True)
        else:
            nc.tensor.matmul(tps[:, sl1, :], tb_sb[:, 2, :], xbf_0[:, (j - 1) ^ 1, :],
                             start=True, stop=False)
            nc.tensor.matmul(tps[:, sl1, :], tb_sb[:, 1, :], xbf_0[:, j ^ 1, :],
                             start=False, stop=False)
            nc.tensor.matmul(tps[:, sl1, :], tb_sb[:, 5, :], corr[:, (j - 1) ^ 1, :],
                             start=False, stop=False)
            nc.tensor.matmul(tps[:, sl1, :], tb_sb[:, 4, :], corr[:, j ^ 1, :],
                             start=False, stop=True)
        if j % 2 == 1:
            sl = slice(j - 1, j + 1)
            nc.vector.tensor_add(out=big_ob[:, sl, :], in0=corr[:, sl, :],
                                 in1=tps[:])
            nc.sync.dma_start(out_d[:, sl, :], big_ob[:, sl, :])


def build_nc(reps=1):
    nc = bacc.Bacc("TRN2", target_bir_lowering=False, debug=False)
    xr8_d = nc.dram_tensor("xr8", [128, NCHUNK, RPC], _f8, kind="ExternalInput")
    xbf_d = nc.dram_tensor("xbf", [128, NCHUNK, RPC], _bf16, kind="ExternalInput")
    e2_d = nc.dram_tensor("e2", [S, 128, 31, 128], _f8, kind="ExternalInput")
    tb_d = nc.dram_tensor("tb", [128, 6, 128], _bf16, kind="ExternalInput")
    out_d = nc.dram_tensor("out", [128, NCHUNK, RPC], _f32, kind="ExternalOutput")

    with tile.TileContext(nc) as tc:
        with (
            tc.tile_pool(name="const", bufs=1) as constp,
            tc.tile_pool(name="xrp", bufs=2) as xrp,
            tc.tile_pool(name="work", bufs=6) as workp,
            tc.tile_pool(name="psum_a", bufs=5, space="PSUM") as psum_a,
            tc.tile_pool(name="psum_t", bufs=2, space="PSUM") as psum_t,
        ):
            pools = (constp, xrp, workp, psum_a, psum_t)
            if reps == 1:
                _emit_body(nc, pools, xr8_d, xbf_d, e2_d, tb_d, out_d)
            else:
                with tc.For_i(0, reps, 1, staggered_reset=True,
                              hint_engines=(mybir.EngineType.PE,)):
                    _emit_body(nc, pools, xr8_d, xbf_d, e2_d, tb_d, out_d)
    nc.compile()
    return nc


_NC_CACHE = {}


def _get_nc(reps=1):
    if reps not in _NC_CACHE:
        _NC_CACHE[reps] = build_nc(reps)
    return _NC_CACHE[reps]


def make_in_maps(x, diagonals):
    c = _circ_cols(diagonals)
    e2 = (c * WSCALE)[:, _E2F_IDX].astype(_np_f8).reshape(S, 128, 31, 128)
    tb = np.concatenate([_TBAND, -_TBAND], axis=1).astype(_np_bf16)  # [128, 6, 128]
    xT = np.asarray(x, dtype=np.float32).reshape(ROWS, L).T  # [L, ROWS]
    in_maps = []
    for i in range(NCORES):
        xs = xT[:, i * RPC : (i + 1) * RPC].reshape(NCHUNK, 128, RPC)
        xs = np.ascontiguousarray(xs[_PERM].transpose(1, 0, 2))  # [128, 16, RPC]
        in_maps.append({
            "xr8": xs.astype(_np_f8),
            "xbf": xs.astype(_np_bf16),
            "e2": e2,
            "tb": tb,
        })
    return in_maps


def gather_out(results):
    parts = []
    for r in results:
        o = r["out"]  # [128, NCHUNK(position order), RPC]
        parts.append(np.ascontiguousarray(
            o.transpose(1, 0, 2)[_PERM]).reshape(L, RPC))
    outT = np.concatenate(parts, axis=1)  # [L, ROWS]
    return np.ascontiguousarray(outT.T).reshape(B, C, L).astype(np.float32)


def kernel(x, diagonals):
    x = np.asarray(x, dtype=np.float32)
    assert x.shape == (B, C, L) and np.asarray(diagonals).shape == (S, L)
    nc = _get_nc(1)
    in_maps = make_in_maps(x, diagonals)
    last_err = None
    for attempt in range(3):
        try:
            res = run_bass_kernel_spmd(nc, in_maps, core_ids=list(range(NCORES)))
            return gather_out(res.results)
        except Exception as ex:  # transient device errors
            last_err = ex
            import time as _time

            _time.sleep(2.0 * (attempt + 1))
    raise last_err
